# revision 1
# baseline (speedup 1.0000x reference)
"""Causal self-attention on 8 Trainium2 NeuronCores (tensor-parallel over heads).

Problem: B=4, T=2048, C=1024, H=16 heads, D=64. fp32 in/out.

Sharding: core i owns heads {2i, 2i+1}. Each core computes its 2 heads'
QKV projection, causal attention, and its 128-column slice of the output
projection, producing a [1024, 8192] partial (transposed, fp32) output.
Host sums the 8 partials and adds b_proj.

All matmul inputs are bf16 (accumulation stays fp32 in PSUM); walrus
emits separate LDWEIGHTS for bf16 so semaphore waits fit, and every
matmul streams at 1 cycle/row.

Layouts:
  - x is passed transposed and pre-cast: xT [C=1024, B*T=8192] bf16.
  - qT/kT per batch: [128 (2 heads x 64), 2048] bf16 = W_slice @ xT,
    bias added by the ScalarE copy out of PSUM.
  - v per batch in natural token-major layout with an appended ones
    column per head: va[t, 130] = [v_h0 | 1 | v_h1 | 1]; the v bias is
    folded into the projection as a K=1 rank-1 update (ones x bias row).
  - scores computed transposed: sT[tk, tq] = kT_tile.T @ qT_chunk; the
    two heads run as concurrent row-tiled matmuls (partitions 0:64 and
    64:128 of the PE array).
  - softmax without max-subtraction (scores ~ N(0,1): exp is safe in
    fp32). Row sums fall out of the PV matmul via the ones column;
    normalization multiplies by a PE-broadcast reciprocal row.
  - causal handling: fully-masked tk tiles are skipped, partial tiles
    compute only their valid columns, and the single 128x128 triangle
    gets an additive -1e30 mask on the PSUM scores before exp.
  - proj: outT[co, t] = wpT_tile.T @ att_chunk -> [1024, 8192] fp32
    partial, transposed back on host.
"""

import numpy as np
import ml_dtypes

import concourse.bass as bass
import concourse.mybir as mybir
from concourse.bass_utils import run_bass_kernel_spmd
from concourse.tile import TileContext

B, T, C = 4, 2048, 1024
H, D = 16, 64
NCORES = 8
BT = B * T              # 8192 tokens total
TB = T                  # tokens per batch
CH = 512                # tq chunk width
NCH = TB // CH          # 4 chunks per batch
KT = C // 128           # 8 contraction tiles for the projections
NTK = TB // 128         # 16 tk tiles per batch
SCALE = 1.0 / np.sqrt(D)

F32 = mybir.dt.float32
BF16 = mybir.dt.bfloat16


def _split_multi_waits(nc):
    """Walrus in this toolchain allows only one sync-wait command per
    instruction; hoist extra waits onto same-engine NOPs placed before."""
    n_split = 0
    for f in nc.m.functions:
        for bb in f.blocks:
            out = []
            for inst in bb.instructions:
                si = inst.sync_info
                if si is not None and si.on_wait and len(si.on_wait) > 1:
                    waits = list(si.on_wait)
                    for k, w in enumerate(waits[:-1]):
                        n_split += 1
                        out.append(
                            mybir.InstNoOp(
                                name=f"{inst.name}-sw{k}",
                                engine=inst.engine,
                                ins=[],
                                outs=[],
                                sync_info=mybir.SyncInfo(
                                    on_wait=[w], on_update=[]
                                ),
                            )
                        )
                    inst.sync_info = mybir.SyncInfo(
                        on_wait=[waits[-1]],
                        on_update=list(si.on_update or []),
                    )
                out.append(inst)
            bb.instructions = out
    return n_split


def build_program(split_waits=True):
    nc = bass.Bass("TRN2", target_bir_lowering=False, debug=False)

    xT = nc.dram_tensor("xT", [C, BT], BF16, kind="ExternalInput").ap()
    wqkvT = nc.dram_tensor("wqkvT", [C, 384], BF16, kind="ExternalInput").ap()
    bqkv = nc.dram_tensor("bqkv", [3, 128, 1], F32, kind="ExternalInput").ap()
    wpT = nc.dram_tensor("wpT", [128, C], BF16, kind="ExternalInput").ap()
    outT = nc.dram_tensor("outT", [C, BT], F32, kind="ExternalOutput").ap()

    with TileContext(nc) as tc:
        with (
            tc.tile_pool(name="consts", bufs=1) as consts,
            tc.tile_pool(name="xin", bufs=3) as xin,
            tc.tile_pool(name="qkv", bufs=2) as qkv,
            tc.tile_pool(name="vall", bufs=2) as vall,
            tc.tile_pool(name="pbuf", bufs=4) as pbuf,
            tc.tile_pool(name="attb", bufs=3) as attb,
            tc.tile_pool(name="small", bufs=4) as small,
            tc.tile_pool(name="outb", bufs=6) as outb,
            tc.tile_pool(name="ps_mm", bufs=3, space="PSUM") as ps_mm,
            tc.tile_pool(name="ps_acc", bufs=2, space="PSUM") as ps_acc,
            tc.tile_pool(name="ps_aux", bufs=2, space="PSUM") as ps_aux,
        ):
            # ---- constants ----
            wqkv_sb = consts.tile([128, KT, 384], BF16, tag="wqkv")
            nc.gpsimd.dma_start(
                out=wqkv_sb, in_=wqkvT.rearrange("(k p) c -> p k c", p=128)
            )
            wp_sb = consts.tile([128, C], BF16, tag="wp")
            nc.gpsimd.dma_start(out=wp_sb, in_=wpT)
            b_sb = []
            for j in range(3):
                bt = consts.tile([128, 1], F32, tag=f"bias{j}")
                nc.gpsimd.dma_start(out=bt, in_=bqkv[j])
                b_sb.append(bt)
            ones_t = consts.tile([1, 128], BF16, tag="onest")
            nc.vector.memset(ones_t, 1.0)
            ident = consts.tile([128, 64], F32, tag="ident")
            from concourse.masks import make_identity
            make_identity(nc, ident[0:64, :])
            make_identity(nc, ident[64:128, :])
            # maskneg[x, y] = 0.0 where x <= y else -1e30 (additive causal mask)
            maskneg = consts.tile([128, 128], F32, tag="tri")
            nc.gpsimd.memset(maskneg, 0.0)
            # keep where (-x + y) >= 0, i.e. x <= y
            nc.gpsimd.affine_select(
                out=maskneg,
                in_=maskneg,
                compare_op=mybir.AluOpType.is_ge,
                fill=-1e30,
                base=0,
                pattern=[[1, 128]],
                channel_multiplier=-1,
            )

            for b in range(B):
                t0 = b * TB
                # ---- phase A: QKV projection for batch b ----
                qT = qkv.tile([128, TB], BF16, tag="q")
                kT = qkv.tile([128, TB], BF16, tag="k")
                vT = qkv.tile([128, TB], F32, tag="v")
                va = vall.tile([128, NTK, 130], BF16, tag="va")
                nc.vector.memset(va[:, :, 64], 1.0)
                nc.vector.memset(va[:, :, 129], 1.0)
                for c in range(NCH):
                    xt = xin.tile([128, KT, CH], BF16, tag="x")
                    nc.gpsimd.dma_start(
                        out=xt,
                        in_=xT[:, t0 + c * CH : t0 + (c + 1) * CH].rearrange(
                            "(k p) t -> p k t", p=128
                        ),
                    )
                    for which, dest in ((0, qT), (1, kT), (2, vT)):
                        ps = ps_mm.tile([128, CH], F32, tag="mm")
                        for k in range(KT):
                            nc.tensor.matmul(
                                ps,
                                lhsT=wqkv_sb[:, k, which * 128 : which * 128 + 128],
                                rhs=xt[:, k, :],
                                start=(k == 0),
                                stop=(k == KT - 1),
                            )
                        nc.scalar.activation(
                            dest[:, c * CH : (c + 1) * CH],
                            ps,
                            mybir.ActivationFunctionType.Identity,
                            bias=b_sb[which],
                        )

                # ---- phase A2: transpose vT into va = [v_h | 1] per head ----
                for j in range(NTK):
                    for h in range(2):
                        tp = ps_aux.tile([128, 64], F32, tag="aux")
                        nc.tensor.transpose(
                            tp,
                            vT[h * 64 : h * 64 + 64, j * 128 : (j + 1) * 128],
                            ident[h * 64 : h * 64 + 64, :],
                        )
                        nc.vector.tensor_copy(va[:, j, h * 65 : h * 65 + 64], tp)

                # ---- phase B: attention, per tq chunk, both heads ----
                for c in range(NCH):
                    tq0 = c * CH
                    njt = 4 * c + 4  # tk tiles 0..njt-1 (rest fully masked)
                    att = attb.tile([128, CH], BF16, tag="att")
                    po = [
                        ps_acc.tile([65, CH], F32, tag="acc", name=f"po{hh}")
                        for hh in range(2)
                    ]
                    for j in range(njt):
                        off = tq0 - 128 * j
                        col0 = max(0, -off)  # first valid column
                        for h in range(2):
                            hs = slice(h * 64, h * 64 + 64)
                            ps = ps_mm.tile([128, CH], F32, tag="mm")
                            nc.tensor.matmul(
                                ps[:, col0:CH],
                                lhsT=kT[hs, j * 128 : (j + 1) * 128],
                                rhs=qT[hs, tq0 + col0 : tq0 + CH],
                                start=True,
                                stop=True,
                            )
                            if off <= 0:
                                nc.vector.tensor_add(
                                    ps[:, col0 : col0 + 128],
                                    ps[:, col0 : col0 + 128],
                                    maskneg,
                                )
                            p = pbuf.tile([128, CH], BF16, tag="p")
                            nc.scalar.activation(
                                p[:, col0:CH],
                                ps[:, col0:CH],
                                mybir.ActivationFunctionType.Exp,
                                scale=float(SCALE),
                            )
                            nc.tensor.matmul(
                                po[h][:, col0:CH],
                                lhsT=va[:, j, h * 65 : h * 65 + 65],
                                rhs=p[:, col0:CH],
                                start=(j == 0),
                                stop=(j == njt - 1),
                            )
                    for h in range(2):
                        nlog = small.tile([1, CH], F32, tag="nlog")
                        nc.scalar.activation(
                            nlog,
                            po[h][64:65, :],
                            mybir.ActivationFunctionType.Ln,
                        )
                        rrow_b = small.tile([1, CH], BF16, tag="rrowb")
                        nc.scalar.activation(
                            rrow_b,
                            nlog,
                            mybir.ActivationFunctionType.Exp,
                            scale=-1.0,
                        )
                        bc = ps_aux.tile([64, CH], F32, tag="aux")
                        nc.tensor.matmul(
                            bc,
                            lhsT=ones_t[:, 0:64],
                            rhs=rrow_b,
                            start=True,
                            stop=True,
                        )
                        raw = small.tile([64, CH], F32, tag="raw")
                        nc.scalar.copy(raw, po[h][0:64, :])
                        nc.vector.tensor_mul(
                            att[h * 64 : h * 64 + 64, :],
                            raw,
                            bc,
                        )

                    # ---- phase C: output projection for this chunk ----
                    for j in range(KT):
                        ps = ps_mm.tile([128, CH], F32, tag="mm")
                        nc.tensor.matmul(
                            ps,
                            lhsT=wp_sb[:, j * 128 : (j + 1) * 128],
                            rhs=att,
                            start=True,
                            stop=True,
                        )
                        ob = outb.tile([128, CH], F32, tag="ob")
                        if j % 2 == 0:
                            nc.scalar.copy(ob, ps)
                        else:
                            nc.vector.tensor_copy(ob, ps)
                        nc.sync.dma_start(
                            out=outT[
                                j * 128 : (j + 1) * 128,
                                t0 + tq0 : t0 + tq0 + CH,
                            ],
                            in_=ob,
                        )
    if split_waits:
        _split_multi_waits(nc)
    return nc


_CACHE = {}
last_run_info = {}


def _get_program():
    if "nc" not in _CACHE:
        _CACHE["nc"] = build_program()
    return _CACHE["nc"]


def _pack_inputs(x, w_qkv, b_qkv, w_proj):
    xT = np.ascontiguousarray(x.reshape(BT, C).T).astype(ml_dtypes.bfloat16)
    in_maps = []
    for i in range(NCORES):
        s = slice(128 * i, 128 * (i + 1))
        wq, wk, wv = w_qkv[0:C][s], w_qkv[C : 2 * C][s], w_qkv[2 * C : 3 * C][s]
        wqkvT = np.ascontiguousarray(np.concatenate([wq, wk, wv], 0).T).astype(
            ml_dtypes.bfloat16
        )
        bq = np.stack(
            [b_qkv[0:C][s], b_qkv[C : 2 * C][s], b_qkv[2 * C : 3 * C][s]]
        )[..., None].astype(np.float32)
        wpT = np.ascontiguousarray(w_proj[:, s].T).astype(ml_dtypes.bfloat16)
        in_maps.append(
            {
                "xT": xT,
                "wqkvT": wqkvT,
                "bqkv": np.ascontiguousarray(bq),
                "wpT": wpT,
            }
        )
    return in_maps


def kernel(x, w_qkv, b_qkv, w_proj, b_proj):
    x = np.asarray(x, dtype=np.float32)
    w_qkv = np.asarray(w_qkv, dtype=np.float32)
    b_qkv = np.asarray(b_qkv, dtype=np.float32)
    w_proj = np.asarray(w_proj, dtype=np.float32)
    b_proj = np.asarray(b_proj, dtype=np.float32)

    nc = _get_program()
    in_maps = _pack_inputs(x, w_qkv, b_qkv, w_proj)
    import os

    trace = bool(int(os.environ.get("KERNEL_TRACE", "0")))
    res = run_bass_kernel_spmd(nc, in_maps, list(range(NCORES)), trace=trace)
    last_run_info["exec_time_ns"] = res.exec_time_ns
    last_run_info["profile_json"] = res.profile_json

    acc = np.zeros((BT, C), dtype=np.float32)
    for i in range(NCORES):
        acc += res.results[i]["outT"].T
    out = acc + b_proj[None, :]
    return out.reshape(B, T, C).astype(np.float32)

